# revision 15
# baseline (speedup 1.0000x reference)
"""Single-head causal attention (B=4, T=2048, C=1024, fp32) on 8 Trainium2 cores.

v5 = v4 + scheduling overhaul (same math: M = Wq^T Wk host-folded so
S = (x_q M) x^T, V projection folded after attention as out = (P x) Wv^T):

- PE warmup: a few dummy N=512 matmuls on memset data run during the initial
  DMA wait, so the tensor-engine p-state ramp (0.65->2.4 GHz over 3us)
  completes before real work.
- Fine-grained, priority-ordered input DMAs: wq in 8 col-block DMAs, xq in
  chunk-group DMAs (needed halves first), xt/xk interleaved as 0.5MB
  col/unit-pair DMAs, masks, then wv. The Q phase starts ~2us in, paced by
  the DMA stream instead of stalling 12us for monolithic loads.
- One-deep S->U software pipeline across the whole pair stream (incl. slot
  boundaries): U(p) is emitted after S(p+1), hiding the exp (Act) latency
  under the next S chain.
- Wide drains: o_ps -> ut via one [128,4x128] Act copy + one DVE copy
  instead of 8 narrow copies; drains overlap the next slot's S chain.
- Vfold(s) emitted inside slot s+1's stream (after S(s+1,1)/U(s+1,0)), giving
  the drain time to land and pushing the wv DMA deadline out of the
  startup-critical window.

Per-core PE floor: Q 65536 + S 73728 + U 73728 + Vfold 65536 = 278528 cycles
= 116us @ 2.4GHz. TimelineSim v4 = 156.3us; v5 targets ~125us.
"""

import os
import sys

import numpy as np

for _p in ("/opt/trn_rl_repo", os.path.expanduser("~/.axon_site/_ro/trn_rl_repo")):
    if os.path.isdir(_p) and _p not in sys.path:
        sys.path.insert(0, _p)

B, T, C = 4, 2048, 1024
NSLOT = 8
SLOT_UNITS = [16 - 2 * s for s in range(NSLOT)]      # [16,14,12,10,8,6,4,2]
ASSIGN = {
    0: [(15 - 2 * s) if s % 2 == 0 else (14 - 2 * s) for s in range(NSLOT)],
    1: [(14 - 2 * s) if s % 2 == 0 else (15 - 2 * s) for s in range(NSLOT)],
}
SCALE = float(C) ** -0.5
NMASK = 2 * NSLOT

_CACHE = {}


def _build_nc(reps=1):
    import concourse.tile as tile
    from concourse import bacc, mybir
    from contextlib import ExitStack

    f32 = mybir.dt.float32
    bf16 = mybir.dt.bfloat16
    Exp = mybir.ActivationFunctionType.Exp
    Copy = mybir.ActivationFunctionType.Copy

    nc = bacc.Bacc("TRN2", target_bir_lowering=False, debug=False)

    # xT: x^T [C,T]; xN: x [T,C]; xqT: this core's q-blocks of x^T gathered
    xT = nc.dram_tensor("xT", [C, T], bf16, kind="ExternalInput").ap()
    xN = nc.dram_tensor("xN", [T, C], bf16, kind="ExternalInput").ap()
    xqT = nc.dram_tensor("xqT", [C, 1024], bf16, kind="ExternalInput").ap()
    # "wqT" carries M = Wq^T @ Wk (host-folded): S = (x_q M) x^T
    wqT = nc.dram_tensor("wqT", [C, C], bf16, kind="ExternalInput").ap()
    wvT = nc.dram_tensor("wvT", [C, C], bf16, kind="ExternalInput").ap()
    # three [128,256] mask patterns (shared unit n1-2, n1-1, extra pair);
    # host phases the content per core-half
    masks = nc.dram_tensor("masks", [3, 128, 256], bf16, kind="ExternalInput").ap()
    out = nc.dram_tensor("out", [1024, C], f32, kind="ExternalOutput").ap()

    with tile.TileContext(nc) as tc:
      for rep in range(reps):
        with ExitStack() as ctx:
            # ---- persistent SBUF arrays ---------------------------------
            kt_pool = ctx.enter_context(tc.tile_pool(name="ktp", bufs=1))
            v_pool = ctx.enter_context(tc.tile_pool(name="vp", bufs=1))
            qt_pool = ctx.enter_context(tc.tile_pool(name="qtp", bufs=1))
            misc_pool = ctx.enter_context(tc.tile_pool(name="miscp", bufs=1))

            xt_all = kt_pool.tile([128, 8 * T], bf16, tag="xta", name="xta")
            xk_all = v_pool.tile([128, 16 * C], bf16, tag="xka", name="xka")
            wv8 = misc_pool.tile([128, 8 * C], bf16, tag="wv8", name="wv8")

            QT = [qt_pool.tile([128, 1024], bf16, tag=f"qt{i}", name=f"qt{i}")
                  for i in range(8)]

            msk = misc_pool.tile([128, 3 * 256], bf16, name="msk")
            # ones doubles as the warmup operand (l-matmuls read [:, 0:2])
            ones = misc_pool.tile([128, 512], bf16, name="ones")
            nc.vector.memset(ones[:], 1.0)

            with tc.tile_pool(name="wp", bufs=1) as w_pool, \
                 tc.tile_pool(name="xp", bufs=1) as x_pool, \
                 tc.psum_pool(name="pproj", bufs=4) as pp:

                wq8 = w_pool.tile([128, 8 * C], bf16, tag="wq8", name="wq8")
                xq = x_pool.tile([128, 8 * 1024], bf16, tag="xq", name="xq")
                wq_v = wq8[:].rearrange("p (a m) -> p a m", a=8)
                xq_v = xq[:].rearrange("p (a m) -> p a m", a=8)

                # -- priority-ordered input DMA stream (one queue, serial) --
                # wq in col-PAIRS (512B DRAM runs keep full DMA rate)
                def wq_pair(cp):
                    nc.sync.dma_start(
                        out=wq_v[:, :, 256 * cp:256 * (cp + 1)],
                        in_=wqT[:, 256 * cp:256 * (cp + 1)].rearrange(
                            "(a p) m -> p a m", p=128))

                def xq_group(g, nchunk, i):
                    nc.sync.dma_start(
                        out=xq_v[:, g:g + nchunk, 512 * i:512 * (i + 1)],
                        in_=xqT[128 * g:128 * (g + nchunk),
                                512 * i:512 * (i + 1)].rearrange(
                            "(a p) m -> p a m", p=128))

                wq_pair(0)
                for g in range(0, 8, 2):
                    xq_group(g, 2, 0)          # i=0 halves, 2 chunks each
                for cp in range(1, 4):
                    wq_pair(cp)
                for g in range(0, 8, 4):
                    xq_group(g, 4, 1)          # i=1 halves, 4 chunks each

                # PE warmup: ramp the p-state while the first DMAs land
                wps = pp.tile([128, 512], f32, tag="pk", name="warm_ps")
                for _ in range(7):
                    nc.tensor.matmul(wps[:], ones[:, 0:128], ones[:],
                                     start=True, stop=True)

                # ---- phase Q: QT[co] = wq.T @ xq -------------------------
                for i in range(2):
                    for co in range(8):
                        ps = pp.tile([128, 512], f32, tag="pk", name=f"qps{i}_{co}")
                        for ci in range(8):
                            nc.tensor.matmul(
                                ps[:],
                                wq8[:, C * ci + 128 * co:C * ci + 128 * (co + 1)],
                                xq[:, 1024 * ci + 512 * i:1024 * ci + 512 * (i + 1)],
                                start=(ci == 0), stop=(ci == 7),
                            )
                        nc.scalar.copy(QT[co][:, 512 * i:512 * (i + 1)], ps[:])

            # attention-phase loads, emitted after the Q-phase loads; the
            # DMA queue drains them in this order while Q computes.
            # xt/xk stream in unit order paced just ahead of slot 0's use;
            # masks are tiny; wv (first needed by Vfold(0) ~15us later)
            # streams last in chunk order.
            xt_v = xt_all[:].rearrange("p (a m) -> p a m", a=8)
            xk_v = xk_all[:].rearrange("p (a m) -> p a m", a=16)
            for t in range(8):
                nc.sync.dma_start(
                    out=xt_v[:, :, 256 * t:256 * (t + 1)],
                    in_=xT[:, 256 * t:256 * (t + 1)].rearrange(
                        "(a p) m -> p a m", p=128))
                nc.sync.dma_start(
                    out=xk_v[:, 2 * t:2 * (t + 1), :],
                    in_=xN[256 * t:256 * (t + 1), :].rearrange(
                        "(a p) m -> p a m", p=128))
            nc.sync.dma_start(
                out=msk[:].rearrange("p (u m) -> p u m", u=3),
                in_=masks[:, :, :].rearrange("u p m -> p u m"),
            )
            for cc in range(8):
                nc.sync.dma_start(
                    out=wv8[:, 1024 * cc:1024 * (cc + 1)],
                    in_=wvT[128 * cc:128 * (cc + 1), :])

            # ---- attention: slot-PAIRED streams --------------------------
            # Adjacent slots (s0 even, s1=s0+1) share k-units 0..n1-1: one S
            # matmul chain covers both slots' q-columns (N=256), so on HW the
            # [128,128] LDWEIGHTS (~53ns with FWL) fully hides under the
            # 106.7ns stream. Slot s0's two extra units run alone (N=128).
            with tc.tile_pool(name="ptp", bufs=3) as pt_pool, \
                 tc.tile_pool(name="outp", bufs=2) as out_pool, \
                 tc.tile_pool(name="utp", bufs=3) as ut_pool, \
                 tc.tile_pool(name="linvp", bufs=2) as linv_pool, \
                 tc.psum_pool(name="sp", bufs=2) as sp, \
                 tc.psum_pool(name="op", bufs=1) as op, \
                 tc.psum_pool(name="op2", bufs=1) as op2, \
                 tc.psum_pool(name="lp", bufs=1) as lp:

                o_ps = {}       # per pair: [128, 8*256] (chunk-major, s0|s1)
                l_ps = {}       # per pair: [128, 4] (s0 at 0:2, s1 at 2:4)
                ut = {}
                linv = {}

                def s_exp(key, src_ps, mask_idx):
                    if mask_idx is not None:
                        sm = pt_pool.tile([128, 256], f32, tag="sm",
                                          name=f"sm{key}")
                        nc.vector.tensor_add(sm[:], src_ps[:],
                                             msk[:, 256 * mask_idx:
                                                 256 * (mask_idx + 1)])
                        src_ps = sm
                    pm = pt_pool.tile([128, 256], bf16, tag="pm",
                                      name=f"pm{key}")
                    nc.scalar.activation(pm[:], src_ps[:], Exp, scale=SCALE)
                    return pm

                def do_S_shared(s0, u, n1):
                    # S for k-unit u over both slots' q-cols [128s0, 128s0+256)
                    s_ps = sp.tile([128, 256], f32, tag="s", name=f"s{s0}_{u}")
                    for ci in range(8):
                        nc.tensor.matmul(
                            s_ps[:],
                            xt_all[:, T * ci + 128 * u:T * ci + 128 * (u + 1)],
                            QT[ci][:, 128 * s0:128 * s0 + 256],
                            start=(ci == 0), stop=(ci == 7),
                        )
                    mask_idx = {n1 - 2: 0, n1 - 1: 1}.get(u)
                    return s_exp(f"{s0}_{u}", s_ps, mask_idx)

                def do_S_extra(s0, n0):
                    # units n0-2, n0-1 over slot s0's q-cols only (v4 layout)
                    s_ps = sp.tile([128, 256], f32, tag="s", name=f"sx{s0}")
                    for d in range(2):
                        j = n0 - 2 + d
                        for ci in range(8):
                            nc.tensor.matmul(
                                s_ps[:, 128 * d:128 * (d + 1)],
                                xt_all[:, T * ci + 128 * j:T * ci + 128 * (j + 1)],
                                QT[ci][:, 128 * s0:128 * (s0 + 1)],
                                start=(ci == 0), stop=(ci == 7),
                            )
                    return s_exp(f"x{s0}", s_ps, 2)

                def do_U_shared(P, u, n1, pm):
                    s0 = P[0]
                    first, last_bank = (u == 0), False
                    for cc in range(8):
                        nc.tensor.matmul(
                            o_ps[P][:, 256 * cc:256 * (cc + 1)],
                            xk_all[:, C * u + 128 * cc:C * u + 128 * (cc + 1)],
                            pm[:], start=(first and cc % 2 == 0), stop=False,
                            skip_group_check=True)
                    nc.tensor.matmul(l_ps[P][:, 0:2], pm[:, 0:128],
                                     ones[:, 0:2], start=first, stop=False,
                                     skip_group_check=True)
                    nc.tensor.matmul(l_ps[P][:, 2:4], pm[:, 128:256],
                                     ones[:, 0:2], start=False,
                                     stop=(u == n1 - 1),
                                     skip_group_check=True)

                def do_U_extra(P, n0, pm):
                    for d in range(2):
                        j = n0 - 2 + d
                        pmu = pm[:, 128 * d:128 * (d + 1)]
                        for cc in range(8):
                            nc.tensor.matmul(
                                o_ps[P][:, 256 * cc:256 * cc + 128],
                                xk_all[:, C * j + 128 * cc:
                                       C * j + 128 * (cc + 1)],
                                pmu, start=False,
                                stop=(d == 1 and cc % 2 == 1),
                                skip_group_check=True)
                        nc.tensor.matmul(l_ps[P][:, 0:2], pmu, ones[:, 0:2],
                                         start=False, stop=(d == 1),
                                         skip_group_check=True)

                def do_drain(P, which):
                    # one slot's 128-col halves of o_ps[P] -> ut[s] (bf16)
                    s = P[which]
                    ut[s] = ut_pool.tile([128, 8 * 128], bf16, tag="ut",
                                         name=f"ut{s}")
                    ut_v = ut[s][:].rearrange("p (a m) -> p a m", a=8)
                    src = o_ps[P][:].rearrange("p (a two m) -> p a two m",
                                               a=8, two=2)[:, :, which, :]
                    nc.scalar.copy(ut_v[:, 0:2, :], src[:, 0:2, :])
                    nc.vector.tensor_copy(ut_v[:, 2:4, :], src[:, 2:4, :])
                    nc.scalar.copy(ut_v[:, 4:6, :], src[:, 4:6, :])
                    nc.vector.tensor_copy(ut_v[:, 6:8, :], src[:, 6:8, :])
                    lv = linv_pool.tile([128, 1], f32, tag="linv",
                                        name=f"linv{s}")
                    nc.vector.reciprocal(lv[:], l_ps[P][:, 2 * which:
                                                        2 * which + 1])
                    linv[s] = lv

                def do_vfold(s):
                    for half in range(2):
                        o2 = op2.tile([128, 512], f32, tag="o2",
                                      name=f"o2_{s}_{half}")
                        for cc in range(8):
                            nc.tensor.matmul(
                                o2[:],
                                ut[s][:, 128 * cc:128 * (cc + 1)],
                                wv8[:, C * cc + 512 * half:
                                    C * cc + 512 * (half + 1)],
                                start=(cc == 0), stop=(cc == 7))
                        o_sb = out_pool.tile([128, 512], f32, tag=f"ost{half}",
                                             name=f"ost{s}_{half}")
                        nc.scalar.activation(o_sb[:], o2[:],
                                             Copy, scale=linv[s][:])
                        nc.sync.dma_start(
                            out=out[128 * s:128 * (s + 1),
                                    512 * half:512 * (half + 1)],
                            in_=o_sb[:],
                        )

                # flat unit stream, one-deep S->U pipeline across pair
                # boundaries; drains get >=1 S chain of slack, vfolds >=2
                # steps. Pair order ends on (2,3) so only vfold(2) trails.
                PAIR_ORDER = [(0, 1), (6, 7), (4, 5), (2, 3)]
                pend = None           # (kind, P, u/n, pm) awaiting U
                pend_vf = []          # [slot, steps_since_drain]

                def pump():
                    # emit U for the pending S; drains when a slot closes
                    if pend is None:
                        return
                    kind, P, arg, pm = pend
                    n0 = SLOT_UNITS[P[0]]
                    if kind == "sh":
                        do_U_shared(P, arg, n0 - 2, pm)
                        if arg == n0 - 3:        # slot s1 closed
                            do_drain(P, 1)
                            pend_vf.append([P[1], 0])
                    else:
                        do_U_extra(P, n0, pm)    # slot s0 closed
                        do_drain(P, 0)
                        pend_vf.append([P[0], 0])
                    for e in pend_vf:
                        e[1] += 1

                def maybe_vfold():
                    if pend_vf and pend_vf[0][1] >= 2:
                        do_vfold(pend_vf.pop(0)[0])

                for P in PAIR_ORDER:
                    s0, s1 = P
                    n0 = SLOT_UNITS[s0]
                    o_ps[P] = op.tile([128, 8 * 256], f32, tag="o",
                                      name=f"o{s0}")
                    l_ps[P] = lp.tile([128, 4], f32, tag="l", name=f"l{s0}")
                    for u in range(n0 - 2):
                        pm = do_S_shared(s0, u, n0 - 2)
                        pump()
                        pend = ("sh", P, u, pm)
                        maybe_vfold()
                    pmx = do_S_extra(s0, n0)
                    pump()
                    pend = ("ex", P, n0, pmx)
                    maybe_vfold()
                # flush
                pump()
                pend = None
                for s_, _ in pend_vf:
                    do_vfold(s_)
    nc.finalize()
    return nc


def _masks_for_half(h):
    # three [128,256] patterns. Slot-pair (s0, s1): shared S covers q-cols
    # [s0|s1]; shared units n1-2, n1-1 are slot s1's diagonal region (idx 0,1,
    # left half always all-pass); the extra pair (units n0-2, n0-1, slot s0
    # only, unit-major) is idx 2.
    #   h=0: g1 = n1-2 -> idx0=[Z|D], idx1=[Z|F];  g0 = n0-1 -> idx2=[Z|D]
    #   h=1: g1 = n1-1 -> idx0=[Z|Z], idx1=[Z|D];  g0 = n0-2 -> idx2=[D|F]
    import ml_dtypes
    k = np.arange(128)[:, None]
    q = np.arange(128)[None, :]
    D = np.where(k <= q, 0.0, -30000.0).astype(np.float32)
    Z = np.zeros((128, 128), np.float32)
    F = np.full((128, 128), -30000.0, np.float32)
    cat = lambda a, b: np.concatenate([a, b], axis=1)
    if h == 0:
        m = np.stack([cat(Z, D), cat(Z, F), cat(Z, D)])
    else:
        m = np.stack([cat(Z, Z), cat(Z, D), cat(D, F)])
    return m.astype(ml_dtypes.bfloat16)


def _get_built():
    if "nc" not in _CACHE:
        _CACHE["nc"] = _build_nc()
        _CACHE["masks"] = {h: _masks_for_half(h) for h in (0, 1)}
    return _CACHE["nc"], _CACHE["masks"]


def make_in_maps(x, Wk, Wq, Wv, mks):
    import ml_dtypes
    bf = ml_dtypes.bfloat16
    x = np.asarray(x, np.float32)
    m_fold = np.asarray(Wq, np.float32).T @ np.asarray(Wk, np.float32)
    wqT = np.ascontiguousarray(m_fold.astype(bf))
    wvT = np.ascontiguousarray(np.asarray(Wv, np.float32).T.astype(bf))

    in_maps = []
    for core in range(8):
        b, h = core // 2, core % 2
        xT_b = np.ascontiguousarray(x[b].T.astype(bf))
        gs = ASSIGN[h]
        xqT = np.ascontiguousarray(
            np.concatenate([xT_b[:, 128 * g:128 * (g + 1)] for g in gs], axis=1)
        )
        xN = np.ascontiguousarray(x[b].astype(bf))
        in_maps.append({
            "xT": xT_b, "xN": xN, "xqT": xqT,
            "wqT": wqT, "wvT": wvT,
            "masks": mks[h],
        })
    return in_maps


def kernel(x, Wk, Wq, Wv, **_ignored):
    from concourse.bass_utils import run_bass_kernel_spmd

    nc, mks = _get_built()
    in_maps = make_in_maps(x, Wk, Wq, Wv, mks)
    res = run_bass_kernel_spmd(nc, in_maps, core_ids=list(range(8)))
    _CACHE["last_res"] = res

    out = np.empty((B, T, C), np.float32)
    for core in range(8):
        b, h = core // 2, core % 2
        o = res.results[core]["out"]
        for s, g in enumerate(ASSIGN[h]):
            out[b, 128 * g:128 * (g + 1), :] = o[128 * s:128 * (s + 1), :]
    return out
